# revision 5
# baseline (speedup 1.0000x reference)
"""ChaoticAttentionMechanism Trainium2 kernel.

Sharding: 8 cores = 4 batches x 2 query-halves (data parallel, no collectives).
Each core computes, for its (batch b, query half h):
  outT[d, q] = (softmax(chaotic) @ V @ Wo.T + bo).T  for its 1024 queries vs 2048 keys
  stats[2, q] = [sum_k Y, sum_k Y^2] where Y = phase_coupling * sync (host-side var)

Key algebra (no trig needed):
  cos(arctan2(im, re)) = re/sqrt(re^2+im^2), sin(...) = im/sqrt(...)
  gamma*sync = U'_Q . U'_K  with U' = phase_proj * sqrt(gamma/512 / (re^2+im^2)) pairwise
  scores = (Q*c) . (K*c) with c = HD^-0.25 folded into projection weights
  bif = beta*(1/4 - (tanh(s)-1/2)^2)

All matmuls run in float32r (TF32-like, full PE rate at >=256 free dim, fp32 bytes).
Orientation: transposed everywhere (contraction/feature on partitions) so softmax
denominators become ones-vector matmuls on the PE and attention needs no transposes.
Softmax skips the max-subtraction: |chaotic| <= ~6 for any plausible input scale,
exp stays comfortably in fp32 range.
"""

import math
from contextlib import ExitStack

import numpy as np

import concourse.bacc as bacc
import concourse.bass as bass
import concourse.mybir as mybir
import concourse.tile as tile
from concourse.bass_utils import run_bass_kernel_spmd

B, S, D = 4, 2048, 1024
HALF = D // 2
NCORES = 8
SQ = S // 2           # queries per core
P = 128
DC = D // P           # 8 feature chunks
NQT = SQ // 512       # 2 query tiles of 512
PASSES = 4
KQ = S // PASSES      # 512 keys per pass
KCP = KQ // P         # 4 key chunks per pass
SYNC_LOSS_WEIGHT = 0.01

F32 = mybir.dt.float32
MM_DT = mybir.dt.float32r
AF = mybir.ActivationFunctionType
OP = mybir.AluOpType


def _load_rows(nc, pool, dram, ncols, dt, name, col0=0, bufs=None, rows=None):
    """Load dram[dc*128:(dc+1)*128, col0:col0+ncols] into tiles [128, ncols]."""
    tiles = []
    for dc in range(rows if rows is not None else DC):
        t = pool.tile([P, ncols], dt, name=f"{name}{dc}", tag=f"{name}{dc}", bufs=bufs)
        nc.sync.dma_start(t[:], dram[dc * P:(dc + 1) * P, col0:col0 + ncols])
        tiles.append(t)
    return tiles


def build(nc: bass.Bass, beta: float, gamma: float):
    assert gamma > 1e-8, "phase_coupling must be positive for this kernel"

    # ---------------- DRAM I/O ----------------
    xT = nc.dram_tensor("xT", [D, S], MM_DT, kind="ExternalInput")
    xq = nc.dram_tensor("xq", [D, SQ], MM_DT, kind="ExternalInput")
    wqT = nc.dram_tensor("wqT", [D, D], MM_DT, kind="ExternalInput")
    wkT = nc.dram_tensor("wkT", [D, D], MM_DT, kind="ExternalInput")
    wvT = nc.dram_tensor("wvT", [D, D], MM_DT, kind="ExternalInput")
    wpT = nc.dram_tensor("wpT", [D, D], MM_DT, kind="ExternalInput")
    woT = nc.dram_tensor("woT", [D, D], MM_DT, kind="ExternalInput")
    bq = nc.dram_tensor("bq", [D, 1], F32, kind="ExternalInput")
    bk = nc.dram_tensor("bk", [D, 1], F32, kind="ExternalInput")
    bp = nc.dram_tensor("bp", [D, 1], F32, kind="ExternalInput")
    bo = nc.dram_tensor("bo", [D, 1], F32, kind="ExternalInput")
    bv_row = nc.dram_tensor("bv_row", [1, D], MM_DT, kind="ExternalInput")
    ones_col_d = nc.dram_tensor("ones_col", [P, 1], MM_DT, kind="ExternalInput")
    ones_row_d = nc.dram_tensor("ones_row", [1, P], MM_DT, kind="ExternalInput")

    outT = nc.dram_tensor("outT", [D, SQ], F32, kind="ExternalOutput")
    stats = nc.dram_tensor("stats", [2, SQ], F32, kind="ExternalOutput")

    # scratch spills (re/im phase chunks share one tensor: rows 0-511 re, 512-1023 im)
    qt_d = nc.dram_tensor("qt_d", [D, SQ], MM_DT, kind="Internal")
    uqt_d = nc.dram_tensor("uqt_d", [D, SQ], MM_DT, kind="Internal")
    kt_d = nc.dram_tensor("kt_d", [D, S], MM_DT, kind="Internal")
    ukt_d = nc.dram_tensor("ukt_d", [D, S], MM_DT, kind="Internal")
    v_d = nc.dram_tensor("v_d", [S, D], MM_DT, kind="Internal")

    sqrt_scale = gamma / 512.0  # U' = pf * Sqrt(recip(nsq) * sqrt_scale)

    with tile.TileContext(nc) as tc, ExitStack() as ctx:
        const = ctx.enter_context(tc.tile_pool(name="const", bufs=1))
        persist = ctx.enter_context(tc.tile_pool(name="persist", bufs=1))

        ones_col = const.tile([P, 1], MM_DT)
        nc.sync.dma_start(ones_col[:], ones_col_d[:, :])
        ones_row = const.tile([1, P], MM_DT)
        nc.sync.dma_start(ones_row[:], ones_row_d[:, :])

        def load_bias(name, dram):
            t = const.tile([P, DC], F32, name=name, tag=name)
            nc.sync.dma_start(t[:], dram.ap().rearrange("(c p) o -> p (c o)", p=P))
            return t

        bq_t = load_bias("bq_t", bq)
        bk_t = load_bias("bk_t", bk)
        bp_t = load_bias("bp_t", bp)
        bo_t = load_bias("bo_t", bo)
        neg_half = const.tile([P, 1], F32)
        nc.vector.memset(neg_half[:], -0.5)

        s_uk = persist.tile([P, DC], F32)
        s_uk_mm = persist.tile([P, DC], MM_DT)
        avt = [persist.tile([P, SQ], MM_DT, name=f"avt{d}", tag=f"avt{d}")
               for d in range(DC)]
        sy_sb = persist.tile([1, SQ], F32)
        syy_acc = persist.tile([1, SQ], F32)
        se_acc = persist.tile([1, SQ], F32)

        # ============ STAGE 1: projections + phase normalization ============

        def proj_psum(ps_pool, w_tiles, w_col0, rhs_tiles, col):
            """psum [128,512] = sum_ki w[ki][:, w_col0:+128].T @ rhs[ki][:, col:+512]"""
            pt = ps_pool.tile([P, 512], F32, name="proj_ps", tag="proj")
            for ki in range(DC):
                nc.tensor.matmul(
                    pt[:], w_tiles[ki][:, w_col0:w_col0 + P],
                    rhs_tiles[ki][:, col:col + 512],
                    start=(ki == 0), stop=(ki == DC - 1))
            return pt

        def project_spill(ps_pool, ev_pool, w_tiles, rhs_tiles, ncols, bias_t,
                          out_dram):
            """out_dram[dc*128.., col] = W.T-proj + bias, evac via ACT."""
            for dc in range(DC):
                for col in range(0, ncols, 512):
                    pt = proj_psum(ps_pool, w_tiles, dc * P, rhs_tiles, col)
                    ob = ev_pool.tile([P, 512], MM_DT, name="p_ev", tag="p_ev")
                    nc.scalar.activation(ob[:], pt[:], AF.Identity,
                                         bias=bias_t[:, dc:dc + 1])
                    nc.sync.dma_start(out_dram[dc * P:(dc + 1) * P, col:col + 512],
                                      ob[:])

        def phase_stage(src_dram, ncols, out_dram, do_rowsum, wp_name):
            """src (QT' or KT') -> phase proj -> pairwise normalize -> U' spill."""
            with tc.tile_pool(name=f"ph_{wp_name}", bufs=1) as sp, \
                 tc.tile_pool(name=f"phw_{wp_name}", bufs=2) as work, \
                 tc.tile_pool(name=f"php_{wp_name}", bufs=4, space="PSUM") as psp:
                wp_t = _load_rows(nc, sp, wpT, D, MM_DT, wp_name)
                src_t = _load_rows(nc, sp, src_dram, ncols, MM_DT, f"{wp_name}src")
                for pc in range(4):  # pair (pc, pc+4)
                    pre = sp.tile([P, ncols], F32, name="pre", tag="pre")
                    pim = sp.tile([P, ncols], F32, name="pim", tag="pim")
                    for col in range(0, ncols, 512):
                        pt = proj_psum(psp, wp_t, pc * P, src_t, col)
                        nc.scalar.activation(pre[:, col:col + 512], pt[:],
                                             AF.Identity, bias=bp_t[:, pc:pc + 1])
                        pt2 = proj_psum(psp, wp_t, (pc + 4) * P, src_t, col)
                        nc.scalar.activation(pim[:, col:col + 512], pt2[:],
                                             AF.Identity, bias=bp_t[:, pc + 4:pc + 5])
                    for col in range(0, ncols, 512):
                        csl = slice(col, col + 512)
                        r2 = work.tile([P, 512], F32, name="r2", tag="n_r2")
                        nc.scalar.activation(r2[:], pre[:, csl], AF.Square)
                        i2 = work.tile([P, 512], F32, name="i2", tag="n_i2")
                        nc.scalar.activation(i2[:], pim[:, csl], AF.Square)
                        nsq = work.tile([P, 512], F32, name="nsq", tag="n_nsq")
                        nc.vector.tensor_add(nsq[:], r2[:], i2[:])
                        rec = work.tile([P, 512], F32, name="rec", tag="n_rec")
                        nc.vector.reciprocal(rec[:], nsq[:])
                        inv = work.tile([P, 512], F32, name="inv", tag="n_inv")
                        nc.scalar.activation(inv[:], rec[:], AF.Sqrt,
                                             scale=float(sqrt_scale))
                        ure = work.tile([P, 512], MM_DT, name="ure", tag="n_ure")
                        nc.vector.tensor_mul(ure[:], pre[:, csl], inv[:])
                        uim = work.tile([P, 512], MM_DT, name="uim", tag="n_uim")
                        nc.vector.tensor_mul(uim[:], pim[:, csl], inv[:])
                        nc.sync.dma_start(out_dram[pc * P:(pc + 1) * P, csl], ure[:])
                        nc.sync.dma_start(out_dram[(pc + 4) * P:(pc + 5) * P, csl],
                                          uim[:])
                        if do_rowsum:
                            sre = work.tile([P, 1], F32, name="sre", tag="n_sre")
                            nc.vector.reduce_sum(sre[:], ure[:],
                                                 axis=mybir.AxisListType.X)
                            sim = work.tile([P, 1], F32, name="sim", tag="n_sim")
                            nc.vector.reduce_sum(sim[:], uim[:],
                                                 axis=mybir.AxisListType.X)
                            if col == 0:
                                nc.vector.tensor_copy(s_uk[:, pc:pc + 1], sre[:])
                                nc.vector.tensor_copy(s_uk[:, pc + 4:pc + 5], sim[:])
                            else:
                                nc.vector.tensor_add(s_uk[:, pc:pc + 1],
                                                     s_uk[:, pc:pc + 1], sre[:])
                                nc.vector.tensor_add(s_uk[:, pc + 4:pc + 5],
                                                     s_uk[:, pc + 4:pc + 5], sim[:])

        # ---- Q' projection ----
        with tc.tile_pool(name="s1q", bufs=1) as sp, \
             tc.tile_pool(name="s1qe", bufs=3) as ev, \
             tc.tile_pool(name="ps1q", bufs=4, space="PSUM") as psq:
            xq_t = _load_rows(nc, sp, xq, SQ, MM_DT, "xq")
            wq_t = _load_rows(nc, sp, wqT, D, MM_DT, "wq")
            project_spill(psq, ev, wq_t, xq_t, SQ, bq_t, qt_d)

        # ---- K' projection ----
        with tc.tile_pool(name="s1k", bufs=1) as sp, \
             tc.tile_pool(name="s1ke", bufs=3) as ev, \
             tc.tile_pool(name="ps1k", bufs=4, space="PSUM") as psk:
            xk_t = _load_rows(nc, sp, xT, S, MM_DT, "xk")
            wk_t = _load_rows(nc, sp, wkT, D, MM_DT, "wk")
            project_spill(psk, ev, wk_t, xk_t, S, bk_t, kt_d)

        # ---- phase stages ----
        phase_stage(qt_d, SQ, uqt_d, False, "wpq")
        phase_stage(kt_d, S, ukt_d, True, "wpk")
        nc.vector.tensor_copy(s_uk_mm[:], s_uk[:])

        # ---- V (natural layout [k, d]; bias bv replicated over partitions) ----
        with tc.tile_pool(name="s1v", bufs=1) as sp, \
             tc.tile_pool(name="s1ve", bufs=3) as ev, \
             tc.tile_pool(name="ps1v", bufs=4, space="PSUM") as psv:
            xv_t = _load_rows(nc, sp, xT, S, MM_DT, "xv")
            wv_t = _load_rows(nc, sp, wvT, D, MM_DT, "wv")
            bvm = sp.tile([1, D], MM_DT)
            nc.sync.dma_start(bvm[:], bv_row[:, :])
            rep_bv = sp.tile([P, D], F32)
            for dh in range(2):
                bp_ps = psv.tile([P, 512], F32, name="bv_ps", tag="proj")
                nc.tensor.matmul(bp_ps[:], ones_row[:],
                                 bvm[:, dh * 512:(dh + 1) * 512],
                                 start=True, stop=True)
                nc.scalar.copy(rep_bv[:, dh * 512:(dh + 1) * 512], bp_ps[:])
            for kc in range(S // P):
                vt = ev.tile([P, D], MM_DT, name="vt", tag="vt")
                for dh in range(2):
                    pt = psv.tile([P, 512], F32, name="v_ps", tag="proj")
                    for ki in range(DC):
                        nc.tensor.matmul(
                            pt[:], xv_t[ki][:, kc * P:(kc + 1) * P],
                            wv_t[ki][:, dh * 512:(dh + 1) * 512],
                            start=(ki == 0), stop=(ki == DC - 1))
                    nc.vector.tensor_add(vt[:, dh * 512:(dh + 1) * 512], pt[:],
                                         rep_bv[:, dh * 512:(dh + 1) * 512])
                nc.sync.dma_start(v_d[kc * P:(kc + 1) * P, :], vt[:])

        # ==================== STAGE 2: attention ====================
        with tc.tile_pool(name="qres", bufs=1) as qres, \
             tc.tile_pool(name="kside", bufs=1) as kside, \
             tc.tile_pool(name="work2", bufs=2) as work, \
             tc.tile_pool(name="epool", bufs=2) as epool, \
             tc.tile_pool(name="small2", bufs=2) as small, \
             tc.tile_pool(name="ps2", bufs=1, space="PSUM") as ps2:
            qt_t = _load_rows(nc, qres, qt_d, SQ, MM_DT, "q2")
            uqt_t = _load_rows(nc, qres, uqt_d, SQ, MM_DT, "uq2")

            # one-shot sum_k Y per q via s_uk
            for qt in range(NQT):
                sy_ps = ps2.tile([1, 512], F32, name="sy_ps", tag="stat", bufs=1)
                for dc in range(DC):
                    nc.tensor.matmul(sy_ps[:], s_uk_mm[:, dc:dc + 1],
                                     uqt_t[dc][:, qt * 512:(qt + 1) * 512],
                                     start=(dc == 0), stop=(dc == DC - 1))
                nc.vector.tensor_copy(sy_sb[:, qt * 512:(qt + 1) * 512], sy_ps[:])

            for p in range(PASSES):
                kt_p = _load_rows(nc, kside, kt_d, KQ, MM_DT, "ktp",
                                  col0=p * KQ, bufs=1)
                ukt_p = _load_rows(nc, kside, ukt_d, KQ, MM_DT, "uktp",
                                   col0=p * KQ, bufs=1)
                v_p = []
                for kc in range(KCP):
                    kg = p * KCP + kc
                    vt = kside.tile([P, D], MM_DT, name=f"vp{kc}", tag=f"vp{kc}",
                                    bufs=1)
                    nc.sync.dma_start(vt[:], v_d[kg * P:(kg + 1) * P, :])
                    v_p.append(vt)

                for qt in range(NQT):
                    qsl = slice(qt * 512, (qt + 1) * 512)
                    syy_ps = ps2.tile([1, 512], F32, name="syy_ps", tag="stat",
                                      bufs=1)
                    se_ps = ps2.tile([1, 512], F32, name="se_ps", tag="stat2",
                                     bufs=1)
                    e_tiles = []
                    for kc in range(KCP):
                        ksl = slice(kc * P, (kc + 1) * P)
                        sS = ps2.tile([P, 512], F32, name="sS", tag="S", bufs=2)
                        for dc in range(DC):
                            nc.tensor.matmul(sS[:], kt_p[dc][:, ksl],
                                             qt_t[dc][:, qsl],
                                             start=(dc == 0), stop=(dc == DC - 1))
                        u = work.tile([P, 512], F32, name="u", tag="u")
                        nc.scalar.activation(u[:], sS[:], AF.Tanh)
                        sq = work.tile([P, 512], F32, name="sq", tag="sq")
                        nc.scalar.activation(sq[:], u[:], AF.Square,
                                             bias=neg_half[:, 0:1])
                        sY = ps2.tile([P, 512], F32, name="sY", tag="Y", bufs=2)
                        for dc in range(DC):
                            nc.tensor.matmul(sY[:], ukt_p[dc][:, ksl],
                                             uqt_t[dc][:, qsl],
                                             start=(dc == 0), stop=(dc == DC - 1))
                        ysq = work.tile([P, 512], MM_DT, name="ysq", tag="ysq")
                        nc.scalar.activation(ysq[:], sY[:], AF.Square)
                        d1 = work.tile([P, 512], F32, name="d1", tag="d1")
                        nc.vector.scalar_tensor_tensor(
                            d1[:], sq[:], -float(beta), sY[:],
                            op0=OP.mult, op1=OP.add)
                        ein = work.tile([P, 512], F32, name="ein", tag="ein")
                        nc.vector.scalar_tensor_tensor(
                            ein[:], sS[:], float(beta) / 4.0, d1[:],
                            op0=OP.add, op1=OP.add)
                        e = epool.tile([P, 512], MM_DT, name="e", tag=f"e{kc}")
                        nc.scalar.activation(e[:], ein[:], AF.Exp)
                        e_tiles.append(e)
                        nc.tensor.matmul(syy_ps[:], ones_col[:], ysq[:],
                                         start=(kc == 0), stop=(kc == KCP - 1))
                        nc.tensor.matmul(se_ps[:], ones_col[:], e[:],
                                         start=(kc == 0), stop=(kc == KCP - 1))
                    # drain per-pass stats
                    if p == 0:
                        nc.vector.tensor_copy(syy_acc[:, qsl], syy_ps[:])
                        nc.vector.tensor_copy(se_acc[:, qsl], se_ps[:])
                    else:
                        nc.vector.tensor_add(syy_acc[:, qsl], syy_acc[:, qsl],
                                             syy_ps[:])
                        nc.vector.tensor_add(se_acc[:, qsl], se_acc[:, qsl],
                                             se_ps[:])
                    # partial attn @ V
                    for dc in range(DC):
                        av_ps = ps2.tile([P, 512], F32, name="av_ps", tag="av",
                                         bufs=2)
                        for kc in range(KCP):
                            nc.tensor.matmul(av_ps[:],
                                             v_p[kc][:, dc * P:(dc + 1) * P],
                                             e_tiles[kc][:],
                                             start=(kc == 0), stop=(kc == KCP - 1))
                        if p == 0:
                            nc.scalar.copy(avt[dc][:, qsl], av_ps[:])
                        else:
                            nc.vector.tensor_add(avt[dc][:, qsl], avt[dc][:, qsl],
                                                 av_ps[:])

            # normalize avt by 1/rowsum(e)
            for qt in range(NQT):
                qsl = slice(qt * 512, (qt + 1) * 512)
                rec = small.tile([1, 512], F32, name="rec2", tag="rec2")
                nc.vector.reciprocal(rec[:], se_acc[:, qsl])
                rec_mm = small.tile([1, 512], MM_DT, name="recmm", tag="recmm")
                nc.vector.tensor_copy(rec_mm[:], rec[:])
                rep_ps = ps2.tile([P, 512], F32, name="rep_ps", tag="av", bufs=2)
                nc.tensor.matmul(rep_ps[:], ones_row[:], rec_mm[:],
                                 start=True, stop=True)
                recipB = small.tile([P, 512], F32, name="recipB", tag="recipB")
                nc.scalar.copy(recipB[:], rep_ps[:])
                for dc in range(DC):
                    nc.vector.tensor_mul(avt[dc][:, qsl], avt[dc][:, qsl],
                                         recipB[:])

        # ==================== STAGE 3: output projection ====================
        with tc.tile_pool(name="s3", bufs=1) as sp3, \
             tc.tile_pool(name="s3ev", bufs=3) as ev3, \
             tc.tile_pool(name="ps3", bufs=4, space="PSUM") as ps3:
            wo_t = _load_rows(nc, sp3, woT, D, MM_DT, "wo")
            for qt in range(NQT):
                qsl = slice(qt * 512, (qt + 1) * 512)
                for ec in range(DC):
                    o_ps = ps3.tile([P, 512], F32, name="o_ps", tag="o")
                    for dc in range(DC):
                        nc.tensor.matmul(
                            o_ps[:], wo_t[dc][:, ec * P:(ec + 1) * P],
                            avt[dc][:, qsl],
                            start=(dc == 0), stop=(dc == DC - 1))
                    o_sb = ev3.tile([P, 512], F32, name="o_sb", tag="o_sb")
                    nc.scalar.activation(o_sb[:], o_ps[:], AF.Identity,
                                         bias=bo_t[:, ec:ec + 1])
                    nc.sync.dma_start(outT[ec * P:(ec + 1) * P, qsl], o_sb[:])
            nc.sync.dma_start(stats[0:1, :], sy_sb[:])
            nc.sync.dma_start(stats[1:2, :], syy_acc[:])

    nc.finalize()
    return nc


_BUILD_CACHE: dict = {}


def _get_nc(beta: float, gamma: float):
    key = (round(beta, 9), round(gamma, 9))
    if key not in _BUILD_CACHE:
        nc = bacc.Bacc("TRN2", target_bir_lowering=False, debug=False,
                       num_devices=NCORES)
        _BUILD_CACHE[key] = build(nc, beta, gamma)
    return _BUILD_CACHE[key]


def make_in_maps(inputs: dict) -> list[dict]:
    f32 = np.float32
    x = np.asarray(inputs["x"], f32)
    c = 1.0 / math.sqrt(math.sqrt(float(D)))  # HD^-0.25; c^2 = 1/sqrt(HD)

    wqT = np.ascontiguousarray(np.asarray(inputs["Wq"], f32).T * f32(c))
    wkT = np.ascontiguousarray(np.asarray(inputs["Wk"], f32).T * f32(c))
    wvT = np.ascontiguousarray(np.asarray(inputs["Wv"], f32).T)
    wpT = np.ascontiguousarray(np.asarray(inputs["Wp"], f32).T * f32(1.0 / c))
    woT = np.ascontiguousarray(np.asarray(inputs["Wo"], f32).T)
    bq = (np.asarray(inputs["bq"], f32) * f32(c)).reshape(D, 1)
    bk = (np.asarray(inputs["bk"], f32) * f32(c)).reshape(D, 1)
    bp = np.asarray(inputs["bp"], f32).reshape(D, 1).copy()
    bo = np.asarray(inputs["bo"], f32).reshape(D, 1).copy()
    bv_row = np.asarray(inputs["bv"], f32).reshape(1, D).copy()
    ones_col = np.ones((P, 1), f32)
    ones_row = np.ones((1, P), f32)

    shared = dict(wqT=wqT, wkT=wkT, wvT=wvT, wpT=wpT, woT=woT,
                  bq=bq, bk=bk, bp=bp, bo=bo, bv_row=bv_row,
                  ones_col=ones_col, ones_row=ones_row)

    in_maps = []
    for core in range(NCORES):
        b, h = core // 2, core % 2
        xTc = np.ascontiguousarray(x[b].T)                       # [D, S]
        xqc = np.ascontiguousarray(xTc[:, h * SQ:(h + 1) * SQ])  # [D, SQ]
        in_maps.append(dict(shared, xT=xTc, xq=xqc))
    return in_maps


def assemble(results: list[dict], gamma: float):
    f32 = np.float32
    out = np.empty((B, S, D), f32)
    var_sum = 0.0
    n = float(S)  # keys per row
    for core in range(NCORES):
        b, h = core // 2, core % 2
        out[b, h * SQ:(h + 1) * SQ, :] = results[core]["outT"].T
        st = results[core]["stats"].astype(np.float64)
        s1 = st[0] / gamma            # sum_k sync
        s2 = st[1] / (gamma * gamma)  # sum_k sync^2
        var = (s2 - s1 * s1 / n) / (n - 1.0)
        var_sum += var.sum()
    loss = SYNC_LOSS_WEIGHT * var_sum / (B * S)
    return out, f32(loss)


def kernel(**inputs):
    beta = float(np.asarray(inputs["bifurcation_param"]))
    gamma = float(np.asarray(inputs["phase_coupling"]))
    nc = _get_nc(beta, gamma)
    in_maps = make_in_maps(inputs)
    res = run_bass_kernel_spmd(nc, in_maps, core_ids=list(range(NCORES)))
    return assemble(res.results, gamma)


# revision 7
# speedup vs baseline: 83.3352x; 83.3352x over previous
"""ChaoticAttentionMechanism Trainium2 kernel.

Sharding: 8 cores = 4 batches x 2 query-halves (data parallel, no collectives).
Each core computes, for its (batch b, query half h):
  outT[d, q] = (softmax(chaotic) @ V @ Wo.T + bo).T  for its 1024 queries vs 2048 keys
  stats[2, q] = [sum_k Y, sum_k Y^2] where Y = phase_coupling * sync (host-side var)

Key algebra (no trig needed):
  cos(arctan2(im, re)) = re/sqrt(re^2+im^2), sin(...) = im/sqrt(...)
  gamma*sync = U'_Q . U'_K  with U' = phase_proj * sqrt(gamma/512 / (re^2+im^2)) pairwise
  scores = (Q*c) . (K*c) with c = HD^-0.25 folded into projection weights
  bif = beta*(1/4 - (tanh(s)-1/2)^2)

All matmuls run in float32r (TF32-like, full PE rate at >=256 free dim, fp32 bytes).
Orientation: transposed everywhere (contraction/feature on partitions) so softmax
denominators become ones-vector matmuls on the PE and attention needs no transposes.
Softmax skips the max-subtraction: |chaotic| <= ~6 for any plausible input scale,
exp stays comfortably in fp32 range.
"""

import math
from contextlib import ExitStack

import numpy as np

import concourse.bacc as bacc
import concourse.bass as bass
import concourse.mybir as mybir
import concourse.tile as tile
from concourse.bass_utils import run_bass_kernel_spmd

B, S, D = 4, 2048, 1024
HALF = D // 2
NCORES = 8
SQ = S // 2           # queries per core
P = 128
DC = D // P           # 8 feature chunks
NQT = SQ // 512       # 2 query tiles of 512
PASSES = 4
KQ = S // PASSES      # 512 keys per pass
KCP = KQ // P         # 4 key chunks per pass
SYNC_LOSS_WEIGHT = 0.01

F32 = mybir.dt.float32
MM_DT = mybir.dt.float32r
AF = mybir.ActivationFunctionType
OP = mybir.AluOpType


def _load_rows(nc, pool, dram, ncols, dt, name, col0=0, bufs=None, rows=None):
    """Load dram[dc*128:(dc+1)*128, col0:col0+ncols] into tiles [128, ncols]."""
    tiles = []
    for dc in range(rows if rows is not None else DC):
        t = pool.tile([P, ncols], dt, name=f"{name}{dc}", tag=f"{name}{dc}", bufs=bufs)
        nc.sync.dma_start(t[:], dram[dc * P:(dc + 1) * P, col0:col0 + ncols])
        tiles.append(t)
    return tiles


def build(nc: bass.Bass, beta: float, gamma: float):
    assert gamma > 1e-8, "phase_coupling must be positive for this kernel"

    # ---------------- DRAM I/O ----------------
    xT = nc.dram_tensor("xT", [D, S], MM_DT, kind="ExternalInput")
    xq = nc.dram_tensor("xq", [D, SQ], MM_DT, kind="ExternalInput")
    wqT = nc.dram_tensor("wqT", [D, D], MM_DT, kind="ExternalInput")
    wkT = nc.dram_tensor("wkT", [D, D], MM_DT, kind="ExternalInput")
    wvT = nc.dram_tensor("wvT", [D, D], MM_DT, kind="ExternalInput")
    wpT = nc.dram_tensor("wpT", [D, D], MM_DT, kind="ExternalInput")
    woT = nc.dram_tensor("woT", [D, D], MM_DT, kind="ExternalInput")
    bq = nc.dram_tensor("bq", [D, 1], F32, kind="ExternalInput")
    bk = nc.dram_tensor("bk", [D, 1], F32, kind="ExternalInput")
    bp = nc.dram_tensor("bp", [D, 1], F32, kind="ExternalInput")
    bo = nc.dram_tensor("bo", [D, 1], F32, kind="ExternalInput")
    bv_row = nc.dram_tensor("bv_row", [1, D], MM_DT, kind="ExternalInput")
    ones_col_d = nc.dram_tensor("ones_col", [P, 1], MM_DT, kind="ExternalInput")
    ones_row_d = nc.dram_tensor("ones_row", [1, P], MM_DT, kind="ExternalInput")

    outT = nc.dram_tensor("outT", [D, SQ], F32, kind="ExternalOutput")
    stats = nc.dram_tensor("stats", [2, SQ], F32, kind="ExternalOutput")

    # scratch spills (re/im phase chunks share one tensor: rows 0-511 re, 512-1023 im)
    kt_d = nc.dram_tensor("kt_d", [D, S], MM_DT, kind="Internal")
    ukt_d = nc.dram_tensor("ukt_d", [D, S], MM_DT, kind="Internal")
    v_d = nc.dram_tensor("v_d", [S, D], MM_DT, kind="Internal")

    sqrt_scale = gamma / 512.0  # U' = pf * Sqrt(recip(nsq) * sqrt_scale)

    with tile.TileContext(nc) as tc, ExitStack() as ctx:
        const = ctx.enter_context(tc.tile_pool(name="const", bufs=1))
        persist = ctx.enter_context(tc.tile_pool(name="persist", bufs=1))

        ones_col = const.tile([P, 1], MM_DT)
        nc.sync.dma_start(ones_col[:], ones_col_d[:, :])
        ones_row = const.tile([1, P], MM_DT)
        nc.sync.dma_start(ones_row[:], ones_row_d[:, :])

        def load_bias(name, dram):
            t = const.tile([P, DC], F32, name=name, tag=name)
            nc.sync.dma_start(t[:], dram.ap().rearrange("(c p) o -> p (c o)", p=P))
            return t

        bq_t = load_bias("bq_t", bq)
        bk_t = load_bias("bk_t", bk)
        bp_t = load_bias("bp_t", bp)
        bo_t = load_bias("bo_t", bo)
        neg_half = const.tile([P, 1], F32)
        nc.vector.memset(neg_half[:], -0.5)

        s_uk = persist.tile([P, DC], F32)
        s_uk_mm = persist.tile([P, DC], MM_DT)
        avt = [persist.tile([P, SQ], MM_DT, name=f"avt{d}", tag=f"avt{d}")
               for d in range(DC)]
        sy_sb = persist.tile([1, SQ], F32)
        syy_acc = persist.tile([1, SQ], F32)
        se_acc = persist.tile([1, SQ], F32)

        # ============ STAGE 1: projections + phase normalization ============

        def proj_psum(ps_pool, w_tiles, w_col0, rhs_tiles, col):
            """psum [128,512] = sum_ki w[ki][:, w_col0:+128].T @ rhs[ki][:, col:+512]"""
            pt = ps_pool.tile([P, 512], F32, name="proj_ps", tag="proj")
            for ki in range(DC):
                nc.tensor.matmul(
                    pt[:], w_tiles[ki][:, w_col0:w_col0 + P],
                    rhs_tiles[ki][:, col:col + 512],
                    start=(ki == 0), stop=(ki == DC - 1))
            return pt

        def project_spill(ps_pool, ev_pool, w_tiles, rhs_tiles, ncols, bias_t,
                          out_dram):
            """out_dram[dc*128.., col] = W.T-proj + bias, evac via ACT."""
            for dc in range(DC):
                for col in range(0, ncols, 512):
                    pt = proj_psum(ps_pool, w_tiles, dc * P, rhs_tiles, col)
                    ob = ev_pool.tile([P, 512], MM_DT, name="p_ev", tag="p_ev")
                    nc.scalar.activation(ob[:], pt[:], AF.Identity,
                                         bias=bias_t[:, dc:dc + 1])
                    nc.sync.dma_start(out_dram[dc * P:(dc + 1) * P, col:col + 512],
                                      ob[:])

        def phase_stage(src_t, ncols, out_dram, out_tiles, do_rowsum, wp_name):
            """src tiles (QT' or KT') -> phase proj -> pairwise normalize -> U'
            written to out_dram (spill) or out_tiles (resident)."""
            with tc.tile_pool(name=f"ph_{wp_name}", bufs=1) as sp, \
                 tc.tile_pool(name=f"phw_{wp_name}", bufs=2) as work, \
                 tc.tile_pool(name=f"php_{wp_name}", bufs=4, space="PSUM") as psp:
                wp_t = _load_rows(nc, sp, wpT, D, MM_DT, wp_name)
                for pc in range(4):  # pair (pc, pc+4)
                    pre = sp.tile([P, ncols], F32, name="pre", tag="pre")
                    pim = sp.tile([P, ncols], F32, name="pim", tag="pim")
                    for col in range(0, ncols, 512):
                        pt = proj_psum(psp, wp_t, pc * P, src_t, col)
                        nc.scalar.activation(pre[:, col:col + 512], pt[:],
                                             AF.Identity, bias=bp_t[:, pc:pc + 1])
                        pt2 = proj_psum(psp, wp_t, (pc + 4) * P, src_t, col)
                        nc.scalar.activation(pim[:, col:col + 512], pt2[:],
                                             AF.Identity, bias=bp_t[:, pc + 4:pc + 5])
                    for col in range(0, ncols, 512):
                        csl = slice(col, col + 512)
                        r2 = work.tile([P, 512], F32, name="r2", tag="n_r2")
                        nc.scalar.activation(r2[:], pre[:, csl], AF.Square)
                        i2 = work.tile([P, 512], F32, name="i2", tag="n_i2")
                        nc.scalar.activation(i2[:], pim[:, csl], AF.Square)
                        nsq = work.tile([P, 512], F32, name="nsq", tag="n_nsq")
                        nc.vector.tensor_add(nsq[:], r2[:], i2[:])
                        rec = work.tile([P, 512], F32, name="rec", tag="n_rec")
                        nc.vector.reciprocal(rec[:], nsq[:])
                        inv = work.tile([P, 512], F32, name="inv", tag="n_inv")
                        nc.scalar.activation(inv[:], rec[:], AF.Sqrt,
                                             scale=float(sqrt_scale))
                        if out_tiles is not None:
                            ure = out_tiles[pc][:, csl]
                            uim = out_tiles[pc + 4][:, csl]
                        else:
                            ure = work.tile([P, 512], MM_DT, name="ure", tag="n_ure")
                            uim = work.tile([P, 512], MM_DT, name="uim", tag="n_uim")
                        nc.vector.tensor_mul(ure[:], pre[:, csl], inv[:])
                        nc.vector.tensor_mul(uim[:], pim[:, csl], inv[:])
                        if out_tiles is None:
                            nc.sync.dma_start(out_dram[pc * P:(pc + 1) * P, csl],
                                              ure[:])
                            nc.sync.dma_start(out_dram[(pc + 4) * P:(pc + 5) * P,
                                                       csl], uim[:])
                        if do_rowsum:
                            sre = work.tile([P, 1], F32, name="sre", tag="n_sre")
                            nc.vector.reduce_sum(sre[:], ure[:],
                                                 axis=mybir.AxisListType.X)
                            sim = work.tile([P, 1], F32, name="sim", tag="n_sim")
                            nc.vector.reduce_sum(sim[:], uim[:],
                                                 axis=mybir.AxisListType.X)
                            if col == 0:
                                nc.vector.tensor_copy(s_uk[:, pc:pc + 1], sre[:])
                                nc.vector.tensor_copy(s_uk[:, pc + 4:pc + 5], sim[:])
                            else:
                                nc.vector.tensor_add(s_uk[:, pc:pc + 1],
                                                     s_uk[:, pc:pc + 1], sre[:])
                                nc.vector.tensor_add(s_uk[:, pc + 4:pc + 5],
                                                     s_uk[:, pc + 4:pc + 5], sim[:])

        # ---- K' projection (resident for phase stage; also spilled) ----
        with tc.tile_pool(name="kkeep", bufs=1) as kkeep:
            kt_res = [kkeep.tile([P, S], MM_DT, name=f"ktr{d}", tag=f"ktr{d}")
                      for d in range(DC)]
            with tc.tile_pool(name="s1k", bufs=1) as sp, \
                 tc.tile_pool(name="ps1k", bufs=4, space="PSUM") as psk:
                wk_t = _load_rows(nc, sp, wkT, D, MM_DT, "wk")
                for half in range(2):
                    c0 = half * (S // 2)
                    xk_t = _load_rows(nc, sp, xT, S // 2, MM_DT, "xk", col0=c0)
                    for dc in range(DC):
                        for col in range(0, S // 2, 512):
                            pt = proj_psum(psk, wk_t, dc * P, xk_t, col)
                            dst = kt_res[dc][:, c0 + col:c0 + col + 512]
                            nc.scalar.activation(dst, pt[:], AF.Identity,
                                                 bias=bk_t[:, dc:dc + 1])
                            nc.sync.dma_start(
                                kt_d[dc * P:(dc + 1) * P, c0 + col:c0 + col + 512],
                                dst)
            phase_stage(kt_res, S, ukt_d, None, True, "wpk")
        nc.vector.tensor_copy(s_uk_mm[:], s_uk[:])

        # ---- V (natural layout [k, d]; bias bv replicated over partitions) ----
        with tc.tile_pool(name="s1v", bufs=1) as sp, \
             tc.tile_pool(name="s1ve", bufs=3) as ev, \
             tc.tile_pool(name="ps1v", bufs=4, space="PSUM") as psv:
            wv_t = _load_rows(nc, sp, wvT, D, MM_DT, "wv")
            bvm = sp.tile([1, D], MM_DT)
            nc.sync.dma_start(bvm[:], bv_row[:, :])
            rep_bv = sp.tile([P, D], F32)
            for dh in range(2):
                bp_ps = psv.tile([P, 512], F32, name="bv_ps", tag="proj")
                nc.tensor.matmul(bp_ps[:], ones_row[:],
                                 bvm[:, dh * 512:(dh + 1) * 512],
                                 start=True, stop=True)
                nc.scalar.copy(rep_bv[:, dh * 512:(dh + 1) * 512], bp_ps[:])
            for half in range(2):
                c0 = half * (S // 2)
                xv_t = _load_rows(nc, sp, xT, S // 2, MM_DT, "xv", col0=c0)
                for kc in range(S // 2 // P):
                    kg = half * (S // 2 // P) + kc
                    vt = ev.tile([P, D], MM_DT, name="vt", tag="vt")
                    for dh in range(2):
                        pt = psv.tile([P, 512], F32, name="v_ps", tag="proj")
                        for ki in range(DC):
                            nc.tensor.matmul(
                                pt[:], xv_t[ki][:, kc * P:(kc + 1) * P],
                                wv_t[ki][:, dh * 512:(dh + 1) * 512],
                                start=(ki == 0), stop=(ki == DC - 1))
                        nc.vector.tensor_add(vt[:, dh * 512:(dh + 1) * 512], pt[:],
                                             rep_bv[:, dh * 512:(dh + 1) * 512])
                    nc.sync.dma_start(v_d[kg * P:(kg + 1) * P, :], vt[:])

        # ---- Q' projection + phase (kept resident into attention) ----
        qkeep = ctx.enter_context(tc.tile_pool(name="qkeep", bufs=1))
        qt_res = [qkeep.tile([P, SQ], MM_DT, name=f"qtr{d}", tag=f"qtr{d}")
                  for d in range(DC)]
        uqt_res = [qkeep.tile([P, SQ], MM_DT, name=f"uqtr{d}", tag=f"uqtr{d}")
                   for d in range(DC)]
        with tc.tile_pool(name="s1q", bufs=1) as sp, \
             tc.tile_pool(name="ps1q", bufs=4, space="PSUM") as psq:
            xq_t = _load_rows(nc, sp, xq, SQ, MM_DT, "xq")
            wq_t = _load_rows(nc, sp, wqT, D, MM_DT, "wq")
            for dc in range(DC):
                for col in range(0, SQ, 512):
                    pt = proj_psum(psq, wq_t, dc * P, xq_t, col)
                    nc.scalar.activation(qt_res[dc][:, col:col + 512], pt[:],
                                         AF.Identity, bias=bq_t[:, dc:dc + 1])
        phase_stage(qt_res, SQ, None, uqt_res, False, "wpq")

        # ==================== STAGE 2: attention ====================
        with tc.tile_pool(name="kside", bufs=1) as kside, \
             tc.tile_pool(name="work2", bufs=2) as work, \
             tc.tile_pool(name="epool", bufs=2) as epool, \
             tc.tile_pool(name="small2", bufs=2) as small, \
             tc.tile_pool(name="ps2", bufs=1, space="PSUM") as ps2:
            # one-shot sum_k Y per q via s_uk
            for qt in range(NQT):
                sy_ps = ps2.tile([1, 512], F32, name="sy_ps", tag="stat", bufs=1)
                for dc in range(DC):
                    nc.tensor.matmul(sy_ps[:], s_uk_mm[:, dc:dc + 1],
                                     uqt_res[dc][:, qt * 512:(qt + 1) * 512],
                                     start=(dc == 0), stop=(dc == DC - 1))
                nc.vector.tensor_copy(sy_sb[:, qt * 512:(qt + 1) * 512], sy_ps[:])

            for p in range(PASSES):
                kt_p = _load_rows(nc, kside, kt_d, KQ, MM_DT, "ktp",
                                  col0=p * KQ, bufs=1)
                ukt_p = _load_rows(nc, kside, ukt_d, KQ, MM_DT, "uktp",
                                   col0=p * KQ, bufs=1)
                v_p = []
                for kc in range(KCP):
                    kg = p * KCP + kc
                    vt = kside.tile([P, D], MM_DT, name=f"vp{kc}", tag=f"vp{kc}",
                                    bufs=1)
                    nc.sync.dma_start(vt[:], v_d[kg * P:(kg + 1) * P, :])
                    v_p.append(vt)

                for qt in range(NQT):
                    qsl = slice(qt * 512, (qt + 1) * 512)
                    syy_ps = ps2.tile([1, 512], F32, name="syy_ps", tag="stat",
                                      bufs=1)
                    se_ps = ps2.tile([1, 512], F32, name="se_ps", tag="stat2",
                                     bufs=1)
                    e_tiles = []
                    for kc in range(KCP):
                        ksl = slice(kc * P, (kc + 1) * P)
                        sS = ps2.tile([P, 512], F32, name="sS", tag="S", bufs=2)
                        for dc in range(DC):
                            nc.tensor.matmul(sS[:], kt_p[dc][:, ksl],
                                             qt_res[dc][:, qsl],
                                             start=(dc == 0), stop=(dc == DC - 1))
                        u = work.tile([P, 512], F32, name="u", tag="u")
                        nc.scalar.activation(u[:], sS[:], AF.Tanh)
                        sq = work.tile([P, 512], F32, name="sq", tag="sq")
                        nc.scalar.activation(sq[:], u[:], AF.Square,
                                             bias=neg_half[:, 0:1])
                        sY = ps2.tile([P, 512], F32, name="sY", tag="Y", bufs=2)
                        for dc in range(DC):
                            nc.tensor.matmul(sY[:], ukt_p[dc][:, ksl],
                                             uqt_res[dc][:, qsl],
                                             start=(dc == 0), stop=(dc == DC - 1))
                        ysq = work.tile([P, 512], MM_DT, name="ysq", tag="ysq")
                        nc.scalar.activation(ysq[:], sY[:], AF.Square)
                        d1 = work.tile([P, 512], F32, name="d1", tag="d1")
                        nc.vector.scalar_tensor_tensor(
                            d1[:], sq[:], -float(beta), sY[:],
                            op0=OP.mult, op1=OP.add)
                        ein = work.tile([P, 512], F32, name="ein", tag="ein")
                        nc.vector.scalar_tensor_tensor(
                            ein[:], sS[:], float(beta) / 4.0, d1[:],
                            op0=OP.add, op1=OP.add)
                        e = epool.tile([P, 512], MM_DT, name="e", tag=f"e{kc}")
                        nc.scalar.activation(e[:], ein[:], AF.Exp)
                        e_tiles.append(e)
                        nc.tensor.matmul(syy_ps[:], ones_col[:], ysq[:],
                                         start=(kc == 0), stop=(kc == KCP - 1))
                        nc.tensor.matmul(se_ps[:], ones_col[:], e[:],
                                         start=(kc == 0), stop=(kc == KCP - 1))
                    # drain per-pass stats
                    if p == 0:
                        nc.vector.tensor_copy(syy_acc[:, qsl], syy_ps[:])
                        nc.vector.tensor_copy(se_acc[:, qsl], se_ps[:])
                    else:
                        nc.vector.tensor_add(syy_acc[:, qsl], syy_acc[:, qsl],
                                             syy_ps[:])
                        nc.vector.tensor_add(se_acc[:, qsl], se_acc[:, qsl],
                                             se_ps[:])
                    # partial attn @ V
                    for dc in range(DC):
                        av_ps = ps2.tile([P, 512], F32, name="av_ps", tag="av",
                                         bufs=2)
                        for kc in range(KCP):
                            nc.tensor.matmul(av_ps[:],
                                             v_p[kc][:, dc * P:(dc + 1) * P],
                                             e_tiles[kc][:],
                                             start=(kc == 0), stop=(kc == KCP - 1))
                        if p == 0:
                            nc.scalar.copy(avt[dc][:, qsl], av_ps[:])
                        else:
                            nc.vector.tensor_add(avt[dc][:, qsl], avt[dc][:, qsl],
                                                 av_ps[:])

            # normalize avt by 1/rowsum(e)
            for qt in range(NQT):
                qsl = slice(qt * 512, (qt + 1) * 512)
                rec = small.tile([1, 512], F32, name="rec2", tag="rec2")
                nc.vector.reciprocal(rec[:], se_acc[:, qsl])
                rec_mm = small.tile([1, 512], MM_DT, name="recmm", tag="recmm")
                nc.vector.tensor_copy(rec_mm[:], rec[:])
                rep_ps = ps2.tile([P, 512], F32, name="rep_ps", tag="av", bufs=2)
                nc.tensor.matmul(rep_ps[:], ones_row[:], rec_mm[:],
                                 start=True, stop=True)
                recipB = small.tile([P, 512], F32, name="recipB", tag="recipB")
                nc.scalar.copy(recipB[:], rep_ps[:])
                for dc in range(DC):
                    nc.vector.tensor_mul(avt[dc][:, qsl], avt[dc][:, qsl],
                                         recipB[:])

        # ==================== STAGE 3: output projection ====================
        with tc.tile_pool(name="s3", bufs=1) as sp3, \
             tc.tile_pool(name="s3ev", bufs=3) as ev3, \
             tc.tile_pool(name="ps3", bufs=4, space="PSUM") as ps3:
            wo_t = _load_rows(nc, sp3, woT, D, MM_DT, "wo")
            for qt in range(NQT):
                qsl = slice(qt * 512, (qt + 1) * 512)
                for ec in range(DC):
                    o_ps = ps3.tile([P, 512], F32, name="o_ps", tag="o")
                    for dc in range(DC):
                        nc.tensor.matmul(
                            o_ps[:], wo_t[dc][:, ec * P:(ec + 1) * P],
                            avt[dc][:, qsl],
                            start=(dc == 0), stop=(dc == DC - 1))
                    o_sb = ev3.tile([P, 512], F32, name="o_sb", tag="o_sb")
                    nc.scalar.activation(o_sb[:], o_ps[:], AF.Identity,
                                         bias=bo_t[:, ec:ec + 1])
                    nc.sync.dma_start(outT[ec * P:(ec + 1) * P, qsl], o_sb[:])
            nc.sync.dma_start(stats[0:1, :], sy_sb[:])
            nc.sync.dma_start(stats[1:2, :], syy_acc[:])

    nc.finalize()
    return nc


_BUILD_CACHE: dict = {}


def _get_nc(beta: float, gamma: float):
    key = (round(beta, 9), round(gamma, 9))
    if key not in _BUILD_CACHE:
        nc = bacc.Bacc("TRN2", target_bir_lowering=False, debug=False,
                       num_devices=NCORES)
        _BUILD_CACHE[key] = build(nc, beta, gamma)
    return _BUILD_CACHE[key]


def make_in_maps(inputs: dict) -> list[dict]:
    f32 = np.float32
    x = np.asarray(inputs["x"], f32)
    c = 1.0 / math.sqrt(math.sqrt(float(D)))  # HD^-0.25; c^2 = 1/sqrt(HD)

    wqT = np.ascontiguousarray(np.asarray(inputs["Wq"], f32).T * f32(c))
    wkT = np.ascontiguousarray(np.asarray(inputs["Wk"], f32).T * f32(c))
    wvT = np.ascontiguousarray(np.asarray(inputs["Wv"], f32).T)
    wpT = np.ascontiguousarray(np.asarray(inputs["Wp"], f32).T * f32(1.0 / c))
    woT = np.ascontiguousarray(np.asarray(inputs["Wo"], f32).T)
    bq = (np.asarray(inputs["bq"], f32) * f32(c)).reshape(D, 1)
    bk = (np.asarray(inputs["bk"], f32) * f32(c)).reshape(D, 1)
    bp = np.asarray(inputs["bp"], f32).reshape(D, 1).copy()
    bo = np.asarray(inputs["bo"], f32).reshape(D, 1).copy()
    bv_row = np.asarray(inputs["bv"], f32).reshape(1, D).copy()
    ones_col = np.ones((P, 1), f32)
    ones_row = np.ones((1, P), f32)

    shared = dict(wqT=wqT, wkT=wkT, wvT=wvT, wpT=wpT, woT=woT,
                  bq=bq, bk=bk, bp=bp, bo=bo, bv_row=bv_row,
                  ones_col=ones_col, ones_row=ones_row)

    in_maps = []
    for core in range(NCORES):
        b, h = core // 2, core % 2
        xTc = np.ascontiguousarray(x[b].T)                       # [D, S]
        xqc = np.ascontiguousarray(xTc[:, h * SQ:(h + 1) * SQ])  # [D, SQ]
        in_maps.append(dict(shared, xT=xTc, xq=xqc))
    return in_maps


def assemble(results: list[dict], gamma: float):
    f32 = np.float32
    out = np.empty((B, S, D), f32)
    var_sum = 0.0
    n = float(S)  # keys per row
    for core in range(NCORES):
        b, h = core // 2, core % 2
        out[b, h * SQ:(h + 1) * SQ, :] = results[core]["outT"].T
        st = results[core]["stats"].astype(np.float64)
        s1 = st[0] / gamma            # sum_k sync
        s2 = st[1] / (gamma * gamma)  # sum_k sync^2
        var = (s2 - s1 * s1 / n) / (n - 1.0)
        var_sum += var.sum()
    loss = SYNC_LOSS_WEIGHT * var_sum / (B * S)
    return out, f32(loss)


def kernel(**inputs):
    beta = float(np.asarray(inputs["bifurcation_param"]))
    gamma = float(np.asarray(inputs["phase_coupling"]))
    nc = _get_nc(beta, gamma)
    in_maps = make_in_maps(inputs)
    res = run_bass_kernel_spmd(nc, in_maps, core_ids=list(range(NCORES)))
    return assemble(res.results, gamma)
